# revision 10
# baseline (speedup 1.0000x reference)
"""Trainium2 Bass kernel for nn_Attention_867583394433 (sparse window attention).

Strategy (8 NeuronCores, pure data parallel over windows B_=256 -> 32/core):
  - Host precomputes the position-MLP -> relative-position-bias table, folds it
    with the additive mask into a multiplicative table EM = exp(rpb + mask)
    (fp16), resident in SBUF (8 masks/core).
  - Device, per window, in transposed score layout S^T[m, n]:
      qk^T + v matmuls -> exp on ScalarE -> P = exp(S^T) * EM split across
      VectorE and GpSimd -> flipped PV (P as stationary operand, out [n, d])
      with the softmax denominator as 1-wide matmuls -> reciprocal + broadcast
      normalize on VectorE -> transpose attn-out back to [c, n] (DMA xbar for
      c 0:128, PE transpose for c 128:192) -> output projection -> DMA out.
  - Biases folded via ones rows; q-scale folded into w_q on the host.
"""

import os

import numpy as np

HEADS = 6
D = 32
C = 192
N = 256
B = 256
NMASK = 64
POS_DIM = 12
EPS = 1e-5
NCORES = 8
WPC = B // NCORES  # 32 windows per core
MPC = NMASK // NCORES  # 8 masks per core
REP = B // NMASK  # 4 windows sharing one mask
FREE = HEADS * 2 * N  # 3072: free layout (head, mtile, n)

# elems of the P=exp(S)*EM multiply done on VectorE (rest on GpSimd)
PM_DVE = int(os.environ.get("PM_DVE", "1536"))

_CACHE = {}


def _win_to_b(core, w):
    """Window order within a core: mask-major.  w = j*REP + k  ->  b."""
    j, k = divmod(w, REP)
    return NMASK * k + MPC * core + j


def _ln_np(x, g, b):
    m = x.mean(-1, keepdims=True)
    v = x.var(-1, keepdims=True)
    return (x - m) / np.sqrt(v + EPS) * g + b


def _pos_bias_host(H, W, pw0, pb0, g1, be1, w1, b1, g2, be2, w2, b2, g3, be3, w3, b3):
    """Replicates the reference position MLP + gather -> rpb [N, N, HEADS]."""
    H = int(H)
    W = int(W)
    ph = np.arange(1 - H, H)
    pw = np.arange(1 - W, W)
    biases = (
        np.stack(np.meshgrid(ph, pw, indexing="ij")).reshape(2, -1).T.astype(np.float32)
    )
    pos = biases @ pw0 + pb0
    pos = np.maximum(_ln_np(pos, g1, be1), 0.0) @ w1 + b1
    pos = np.maximum(_ln_np(pos, g2, be2), 0.0) @ w2 + b2
    pos = np.maximum(_ln_np(pos, g3, be3), 0.0) @ w3 + b3
    coords = np.stack(np.meshgrid(np.arange(H), np.arange(W), indexing="ij")).reshape(
        2, -1
    )
    rel = coords[:, :, None] - coords[:, None, :]
    rpi = (rel[0] + H - 1) * (2 * W - 1) + (rel[1] + W - 1)
    return pos[rpi]  # [N, N, HEADS] fp32


def _build_nc(repeat=1):
    import concourse.tile as tile
    from concourse import bacc, mybir

    FP = mybir.dt.float32
    BF = mybir.dt.float16
    EXP = mybir.ActivationFunctionType.Exp
    MUL = mybir.AluOpType.mult

    nc = bacc.Bacc("TRN2", target_bir_lowering=False, debug=False)
    xt_d = nc.dram_tensor("xt", [WPC, 128, 2, N], BF, kind="ExternalInput")
    em_d = nc.dram_tensor("em", [MPC, 128, FREE], BF, kind="ExternalInput")
    wqk_d = nc.dram_tensor("wqk", [193, 512], BF, kind="ExternalInput")
    wv_d = nc.dram_tensor("wv", [193, C], BF, kind="ExternalInput")
    wp_d = nc.dram_tensor("wp", [193, C], BF, kind="ExternalInput")
    id_d = nc.dram_tensor("ident", [128, 128], BF, kind="ExternalInput")
    y_d = nc.dram_tensor("y", [WPC, 128, 2, C], FP, kind="ExternalOutput")

    with tile.TileContext(nc) as tc:
        with (
            tc.tile_pool(name="const", bufs=1) as cpool,
            tc.tile_pool(name="win", bufs=int(os.environ.get("WBUFS", "2"))) as wpool,
            tc.tile_pool(name="big", bufs=int(os.environ.get("BBUFS", "2"))) as bpool,
            tc.tile_pool(name="ps_sc", bufs=2, space="PSUM") as ps_sc,
            tc.tile_pool(name="ps_py", bufs=2, space="PSUM") as ps_py,
        ):
            # ---- resident constants ----
            em_sb = cpool.tile([128, MPC, FREE], BF)
            em_loaded = set()
            wqk_sb = cpool.tile([128, 2, 512], BF)
            nc.sync.dma_start(wqk_sb[:, 0, :], wqk_d[0:128, :])
            nc.sync.dma_start(wqk_sb[0:65, 1, :], wqk_d[128:193, :])
            wv_sb = cpool.tile([128, 2, C], BF)
            nc.sync.dma_start(wv_sb[:, 0, :], wv_d[0:128, :])
            nc.sync.dma_start(wv_sb[0:65, 1, :], wv_d[128:193, :])
            wp_sb = cpool.tile([128, 2, C], BF)
            nc.sync.dma_start(wp_sb[:, 0, :], wp_d[0:128, :])
            nc.sync.dma_start(wp_sb[0:65, 1, :], wp_d[128:193, :])
            ident = cpool.tile([128, 128], BF)
            nc.sync.dma_start(ident[:], id_d[:, :])
            ones1 = cpool.tile([128, 1], BF)
            nc.gpsimd.memset(ones1[:], 1.0)
            # attn-out^T double buffers; row 64 of the hi tile is the ones row
            aoTa = [cpool.tile([128, 2, 128], BF, name=f"aoTa{k}", tag=f"aoTa{k}") for k in range(2)]
            aoTb = [cpool.tile([128, 2, 128], BF, name=f"aoTb{k}", tag=f"aoTb{k}") for k in range(2)]
            for k in range(2):
                nc.gpsimd.memset(aoTb[k][64:65, :, :], 1.0)

            # scores head -> (qk m-tile, partition row) maps (q/k row-aligned)
            q_loc = [(0, 32 * h) for h in range(4)] + [(2, 32 * (h - 4)) for h in (4, 5)]
            k_loc = [(1, 32 * h) for h in range(4)] + [(3, 32 * (h - 4)) for h in (4, 5)]

            def window(w):
                j = w // REP
                if j not in em_loaded:
                    em_loaded.add(j)
                    nc.sync.dma_start(em_sb[:, j, :], em_d[j])
                asel = w % 2

                # ---- x^T load (single DMA; rows 128..192 + ones row in chunk 1)
                xa = wpool.tile([128, 2, N], BF, tag="xa")
                nc.sync.dma_start(xa[:], xt_d[w])

                # ---- qk^T matmuls -> psum -> sbuf fp16
                qkps = ps_sc.tile([128, 4, N], FP, tag="sc")
                for m in range(4):
                    nc.tensor.matmul(
                        qkps[:, m, :],
                        wqk_sb[:, 0, 128 * m : 128 * (m + 1)],
                        xa[:, 0, :],
                        start=True,
                        stop=False,
                    )
                    nc.tensor.matmul(
                        qkps[:, m, :],
                        wqk_sb[0:65, 1, 128 * m : 128 * (m + 1)],
                        xa[0:65, 1, :],
                        start=False,
                        stop=True,
                    )
                qkT = wpool.tile([128, 4, N], BF, tag="qkT")
                nc.vector.tensor_copy(qkT[:], qkps[:])

                # ---- v matmuls (v in [m, c] layout) -> psum -> sbuf fp16
                vps = ps_sc.tile([128, 2, C], FP, tag="sc")
                for mt in range(2):
                    nc.tensor.matmul(
                        vps[:, mt, :],
                        xa[:, 0, 128 * mt : 128 * (mt + 1)],
                        wv_sb[:, 0, :],
                        start=True,
                        stop=False,
                    )
                    nc.tensor.matmul(
                        vps[:, mt, :],
                        xa[0:65, 1, 128 * mt : 128 * (mt + 1)],
                        wv_sb[0:65, 1, :],
                        start=False,
                        stop=True,
                    )
                vsb = wpool.tile([128, 2, C], BF, tag="vsb")
                nc.vector.tensor_copy(vsb[:], vps[:])

                # ---- scores S^T + exp, 2 chunks of 6 (h, mt) slabs
                es = bpool.tile([128, FREE], BF, tag="es")
                for ch in range(2):
                    scps = ps_sc.tile([128, 6, N], FP, tag="sc")
                    for kk in range(6):
                        h, mt = divmod(6 * ch + kk, 2)
                        qt, qr = q_loc[h]
                        kt, kr = k_loc[h]
                        nc.tensor.matmul(
                            scps[:, kk, :],
                            qkT[kr : kr + 32, kt, 128 * mt : 128 * (mt + 1)],
                            qkT[qr : qr + 32, qt, :],
                            start=True,
                            stop=True,
                            tile_position=(kr, 0),
                        )
                    nc.scalar.activation(
                        es[:, 1536 * ch : 1536 * (ch + 1)], scps[:], EXP
                    )

                # ---- P = exp(S^T) * EM, split DVE / GpSimd
                p_t = bpool.tile([128, FREE], BF, tag="P")
                emj = em_sb[:, j, :]
                if PM_DVE > 0:
                    nc.vector.tensor_tensor(
                        p_t[:, 0:PM_DVE], es[:, 0:PM_DVE], emj[:, 0:PM_DVE], MUL
                    )
                if PM_DVE < FREE:
                    nc.gpsimd.tensor_tensor(
                        p_t[:, PM_DVE:FREE], es[:, PM_DVE:FREE], emj[:, PM_DVE:FREE], MUL
                    )

                # ---- flipped PV + denominator:  out [n, 6*(32+1)] per n-chunk
                pv = ps_py.tile([128, 2, 6, 33], FP, tag="py")
                for h in range(HEADS):
                    for nt in range(2):
                        # each accumulation group must complete before the next
                        # starts: a start=True matmul clears has_written for
                        # the whole PSUM bank
                        for mt in range(2):
                            o = 512 * h + 256 * mt + 128 * nt
                            nc.tensor.matmul(
                                pv[:, nt, h, 0:32],
                                p_t[:, o : o + 128],
                                vsb[:, mt, 32 * h : 32 * (h + 1)],
                                start=(mt == 0),
                                stop=(mt == 1),
                            )
                        for mt in range(2):
                            o = 512 * h + 256 * mt + 128 * nt
                            nc.tensor.matmul(
                                pv[:, nt, h, 32:33],
                                p_t[:, o : o + 128],
                                ones1[:],
                                start=(mt == 0),
                                stop=(mt == 1),
                            )

                # ---- normalize: aout[n, h, d] = pv * (1/den) broadcast over d
                ivd = wpool.tile([128, 2, 6], FP, tag="ivd")
                nc.vector.reciprocal_approx_fast(ivd[:], pv[:, :, :, 32])
                aout = wpool.tile([128, 2, 6, 32], BF, tag="aout")
                nc.vector.tensor_tensor(
                    aout[:],
                    pv[:, :, :, 0:32],
                    ivd[:].unsqueeze(3).broadcast_to([128, 2, 6, 32]),
                    MUL,
                )
                av = aout[:].rearrange("p t h d -> p t (h d)")

                # ---- transpose attn-out to [c, n]: DMA xbar (c 0:128) + PE (c 128:192)
                for nt in range(2):
                    nc.scalar.dma_start_transpose(
                        aoTa[asel][:, nt, :], av[:, nt, 0:128]
                    )
                tp = ps_py.tile([64, 2, 128], BF, tag="py")
                for nt in range(2):
                    nc.tensor.transpose(tp[:, nt, :], av[:, nt, 128:192], ident[:])
                nc.vector.tensor_copy(aoTb[asel][0:64, :, :], tp[:])

                # ---- output projection y[n, c] and store
                yps = ps_py.tile([128, 2, C], FP, tag="py")
                for nt in range(2):
                    nc.tensor.matmul(
                        yps[:, nt, :],
                        aoTa[asel][:, nt, :],
                        wp_sb[:, 0, :],
                        start=True,
                        stop=False,
                    )
                    nc.tensor.matmul(
                        yps[:, nt, :],
                        aoTb[asel][0:65, nt, :],
                        wp_sb[0:65, 1, :],
                        start=False,
                        stop=True,
                    )
                ysb = wpool.tile([128, 2, C], FP, tag="ysb")
                dbg = os.environ.get("DBG_STAGE", "")
                if not dbg:
                    nc.scalar.copy(ysb[:], yps[:])
                else:
                    nc.gpsimd.memset(ysb[:], 0.0)
                    if dbg == "es0":
                        nc.vector.tensor_copy(
                            ysb[:].rearrange("p a b -> p (a b)"), es[:, 0:384]
                        )
                    elif dbg == "p0":
                        nc.vector.tensor_copy(
                            ysb[:].rearrange("p a b -> p (a b)"), p_t[:, 0:384]
                        )
                    elif dbg == "pv":
                        nc.vector.tensor_copy(
                            ysb[:].rearrange("p a b -> p (a b)"),
                            pv[:, :, :, 0:32].rearrange("p t h d -> p (t h d)"),
                        )
                    elif dbg == "den":
                        nc.vector.tensor_copy(
                            ysb[:, 0, 0:12], pv[:, :, :, 32].rearrange("p t h -> p (t h)")
                        )
                    elif dbg == "aout":
                        nc.vector.tensor_copy(
                            ysb[:].rearrange("p a (h d) -> p a h d", h=6), aout[:]
                        )
                    elif dbg == "aoTa":
                        nc.vector.tensor_copy(ysb[:, :, 0:128], aoTa[asel][:])
                    elif dbg == "aoTb":
                        nc.vector.tensor_copy(ysb[:, :, 0:128], aoTb[asel][:])
                    elif dbg == "qkT":
                        nc.vector.tensor_copy(
                            ysb[:].rearrange("p a b -> p (a b)"),
                            qkT[:, 0:2, 0:192].rearrange("p a b -> p (a b)"),
                        )
                    elif dbg == "ivd":
                        nc.vector.tensor_copy(
                            ysb[:, 0, 0:12], ivd[:].rearrange("p t h -> p (t h)")
                        )
                    elif dbg == "vsb":
                        nc.vector.tensor_copy(ysb[:], vsb[:])
                nc.sync.dma_start(y_d[w], ysb[:])

            for rep in range(repeat):
                for it in range(WPC):
                    window(it)

    nc.compile()
    return nc


def _prep_inputs(inputs):
    x = np.asarray(inputs["x"], np.float32)
    mask = np.asarray(inputs["mask"], np.float32)
    w_qkv = np.asarray(inputs["w_qkv"], np.float32)
    b_qkv = np.asarray(inputs["b_qkv"], np.float32)
    w_proj = np.asarray(inputs["w_proj"], np.float32)
    b_proj = np.asarray(inputs["b_proj"], np.float32)
    H, W = int(inputs["H"]), int(inputs["W"])

    scale = float(D) ** -0.5
    rpb = _pos_bias_host(
        H,
        W,
        *[
            np.asarray(inputs[k], np.float32)
            for k in (
                "pw0",
                "pb0",
                "g1",
                "be1",
                "w1",
                "b1",
                "g2",
                "be2",
                "w2",
                "b2",
                "g3",
                "be3",
                "w3",
                "b3",
            )
        ],
    )

    # EM[mb, p, h*512 + mt*256 + n] = exp(mask[mb, n, m] + rpb[n, m, h]), m = mt*128+p
    bias = mask.transpose(0, 2, 1)[:, None] + rpb.transpose(2, 1, 0)[None]
    em = np.exp(bias)  # [64, 6, 256(m), 256(n)]
    em = em.reshape(NMASK, HEADS, 2, 128, N).transpose(0, 3, 1, 2, 4)
    em = np.ascontiguousarray(em.reshape(NMASK, 128, FREE)).astype(np.float16)

    # packed/augmented weights
    wq = np.vstack([w_qkv[:, 0:C] * scale, (b_qkv[0:C] * scale)[None]])  # [193, 192]
    wk = np.vstack([w_qkv[:, C : 2 * C], b_qkv[C : 2 * C][None]])
    mmdt = np.float16
    wqk = np.zeros((193, 512), np.float32)
    wqk[:, 0:128] = wq[:, 0:128]
    wqk[:, 128:256] = wk[:, 0:128]
    wqk[:, 256:320] = wq[:, 128:192]
    wqk[:, 384:448] = wk[:, 128:192]
    wqk = wqk.astype(mmdt)
    wv = np.ascontiguousarray(np.vstack([w_qkv[:, 2 * C :], b_qkv[2 * C :][None]])).astype(mmdt)
    wp = np.ascontiguousarray(np.vstack([w_proj, b_proj[None]])).astype(mmdt)
    ident = np.eye(128, dtype=mmdt)

    # per-core x^T with ones row, padded to [B, 128, 2, N] for 1-DMA loads
    xt_aug = np.zeros((B, 128, 2, N), mmdt)
    xT = x.transpose(0, 2, 1)  # [B, C, N]
    xt_aug[:, :, 0, :] = xT[:, 0:128, :]
    xt_aug[:, 0:64, 1, :] = xT[:, 128:192, :]
    xt_aug[:, 64, 1, :] = 1.0

    in_maps = []
    for core in range(NCORES):
        bs = [_win_to_b(core, w) for w in range(WPC)]
        in_maps.append(
            {
                "xt": np.ascontiguousarray(xt_aug[bs]),
                "em": np.ascontiguousarray(em[MPC * core : MPC * (core + 1)]),
                "wqk": wqk,
                "wv": wv,
                "wp": wp,
                "ident": ident,
            }
        )
    return in_maps


def _assemble(results):
    out = np.empty((B, N, C), np.float32)
    for core in range(NCORES):
        y = results[core]["y"]  # [WPC, 128, 2, C]
        for w in range(WPC):
            b = _win_to_b(core, w)
            out[b] = y[w].transpose(1, 0, 2).reshape(N, C)
    return out


def run(inputs, trace=False):
    from concourse.bass_utils import run_bass_kernel_spmd

    if "nc" not in _CACHE:
        _CACHE["nc"] = _build_nc()
    in_maps = _prep_inputs(inputs)
    res = run_bass_kernel_spmd(
        _CACHE["nc"],
        in_maps,
        core_ids=list(range(NCORES)),
        trace=trace,
        trace_cores=[0] if trace else None,
    )
    return _assemble(res.results), res


def get_nc():
    if "nc" not in _CACHE:
        _CACHE["nc"] = _build_nc()
    return _CACHE["nc"]


def kernel(**inputs):
    out, _ = run(inputs, trace=bool(int(os.environ.get("KERNEL_TRACE", "0"))))
    return out


# revision 11
# speedup vs baseline: 1.0647x; 1.0647x over previous
"""Trainium2 Bass kernel for nn_Attention_867583394433 (sparse window attention).

Strategy (8 NeuronCores, pure data parallel over windows B_=256 -> 32/core):
  - Host precomputes the position-MLP -> relative-position-bias table, folds it
    with the additive mask into a multiplicative table EM = exp(rpb + mask)
    (fp16), resident in SBUF (8 masks/core).
  - Device, per window, in transposed score layout S^T[m, n]:
      qk^T + v matmuls -> exp on ScalarE -> P = exp(S^T) * EM split across
      VectorE and GpSimd -> flipped PV (P as stationary operand, out [n, d])
      with the softmax denominator as 1-wide matmuls -> reciprocal + broadcast
      normalize on VectorE -> transpose attn-out back to [c, n] (DMA xbar for
      c 0:128, PE transpose for c 128:192) -> output projection -> DMA out.
  - Biases folded via ones rows; q-scale folded into w_q on the host.
"""

import os

import numpy as np

HEADS = 6
D = 32
C = 192
N = 256
B = 256
NMASK = 64
POS_DIM = 12
EPS = 1e-5
NCORES = 8
WPC = B // NCORES  # 32 windows per core
MPC = NMASK // NCORES  # 8 masks per core
REP = B // NMASK  # 4 windows sharing one mask
FREE = HEADS * 2 * N  # 3072: free layout (head, mtile, n)

# elems of the P=exp(S)*EM multiply done on VectorE (rest on GpSimd)
PM_DVE = int(os.environ.get("PM_DVE", "1536"))

_CACHE = {}


def _win_to_b(core, w):
    """Window order within a core: mask-major.  w = j*REP + k  ->  b."""
    j, k = divmod(w, REP)
    return NMASK * k + MPC * core + j


def _ln_np(x, g, b):
    m = x.mean(-1, keepdims=True)
    v = x.var(-1, keepdims=True)
    return (x - m) / np.sqrt(v + EPS) * g + b


def _pos_bias_host(H, W, pw0, pb0, g1, be1, w1, b1, g2, be2, w2, b2, g3, be3, w3, b3):
    """Replicates the reference position MLP + gather -> rpb [N, N, HEADS]."""
    H = int(H)
    W = int(W)
    ph = np.arange(1 - H, H)
    pw = np.arange(1 - W, W)
    biases = (
        np.stack(np.meshgrid(ph, pw, indexing="ij")).reshape(2, -1).T.astype(np.float32)
    )
    pos = biases @ pw0 + pb0
    pos = np.maximum(_ln_np(pos, g1, be1), 0.0) @ w1 + b1
    pos = np.maximum(_ln_np(pos, g2, be2), 0.0) @ w2 + b2
    pos = np.maximum(_ln_np(pos, g3, be3), 0.0) @ w3 + b3
    coords = np.stack(np.meshgrid(np.arange(H), np.arange(W), indexing="ij")).reshape(
        2, -1
    )
    rel = coords[:, :, None] - coords[:, None, :]
    rpi = (rel[0] + H - 1) * (2 * W - 1) + (rel[1] + W - 1)
    return pos[rpi]  # [N, N, HEADS] fp32


def _build_nc(repeat=1):
    import concourse.tile as tile
    from concourse import bacc, mybir

    FP = mybir.dt.float32
    BF = mybir.dt.float16
    EXP = mybir.ActivationFunctionType.Exp
    MUL = mybir.AluOpType.mult

    nc = bacc.Bacc("TRN2", target_bir_lowering=False, debug=False)
    xt_d = nc.dram_tensor("xt", [WPC, 128, 2, N], BF, kind="ExternalInput")
    em_d = nc.dram_tensor("em", [MPC, 128, FREE], BF, kind="ExternalInput")
    wqk_d = nc.dram_tensor("wqk", [193, 512], BF, kind="ExternalInput")
    wv_d = nc.dram_tensor("wv", [193, C], BF, kind="ExternalInput")
    wp_d = nc.dram_tensor("wp", [193, C], BF, kind="ExternalInput")
    id_d = nc.dram_tensor("ident", [128, 128], BF, kind="ExternalInput")
    y_d = nc.dram_tensor("y", [WPC, 128, 2, C], FP, kind="ExternalOutput")

    with tile.TileContext(nc) as tc:
        with (
            tc.tile_pool(name="const", bufs=1) as cpool,
            tc.tile_pool(name="win", bufs=int(os.environ.get("WBUFS", "3"))) as wpool,
            tc.tile_pool(name="big", bufs=int(os.environ.get("BBUFS", "3"))) as bpool,
            tc.tile_pool(name="ps_sc", bufs=int(os.environ.get("SCBUFS", "3")), space="PSUM") as ps_sc,
            tc.tile_pool(name="ps_py", bufs=2, space="PSUM") as ps_py,
        ):
            # ---- resident constants ----
            em_sb = cpool.tile([128, MPC, FREE], BF)
            em_loaded = set()
            wqk_sb = cpool.tile([128, 2, 512], BF)
            nc.sync.dma_start(wqk_sb[:, 0, :], wqk_d[0:128, :])
            nc.sync.dma_start(wqk_sb[0:65, 1, :], wqk_d[128:193, :])
            wv_sb = cpool.tile([128, 2, C], BF)
            nc.sync.dma_start(wv_sb[:, 0, :], wv_d[0:128, :])
            nc.sync.dma_start(wv_sb[0:65, 1, :], wv_d[128:193, :])
            wp_sb = cpool.tile([128, 2, C], BF)
            nc.sync.dma_start(wp_sb[:, 0, :], wp_d[0:128, :])
            nc.sync.dma_start(wp_sb[0:65, 1, :], wp_d[128:193, :])
            ident = cpool.tile([128, 128], BF)
            nc.sync.dma_start(ident[:], id_d[:, :])
            ones1 = cpool.tile([128, 1], BF)
            nc.gpsimd.memset(ones1[:], 1.0)
            # attn-out^T double buffers; row 64 of the hi tile is the ones row
            NAOT = int(os.environ.get("NAOT", "3"))
            aoTa = [cpool.tile([128, 2, 128], BF, name=f"aoTa{k}", tag=f"aoTa{k}") for k in range(NAOT)]
            aoTb = [cpool.tile([128, 2, 128], BF, name=f"aoTb{k}", tag=f"aoTb{k}") for k in range(NAOT)]
            for k in range(NAOT):
                nc.gpsimd.memset(aoTb[k][64:65, :, :], 1.0)

            # scores head -> (qk m-tile, partition row) maps (q/k row-aligned)
            q_loc = [(0, 32 * h) for h in range(4)] + [(2, 32 * (h - 4)) for h in (4, 5)]
            k_loc = [(1, 32 * h) for h in range(4)] + [(3, 32 * (h - 4)) for h in (4, 5)]

            def window(w):
                j = w // REP
                if j not in em_loaded:
                    em_loaded.add(j)
                    nc.sync.dma_start(em_sb[:, j, :], em_d[j])
                asel = w % NAOT

                # ---- x^T load (single DMA; rows 128..192 + ones row in chunk 1)
                xa = wpool.tile([128, 2, N], BF, tag="xa")
                nc.sync.dma_start(xa[:], xt_d[w])

                # ---- qk^T matmuls -> psum -> sbuf fp16
                qkps = ps_sc.tile([128, 4, N], FP, tag="sc")
                for m in range(4):
                    nc.tensor.matmul(
                        qkps[:, m, :],
                        wqk_sb[:, 0, 128 * m : 128 * (m + 1)],
                        xa[:, 0, :],
                        start=True,
                        stop=False,
                    )
                    nc.tensor.matmul(
                        qkps[:, m, :],
                        wqk_sb[0:65, 1, 128 * m : 128 * (m + 1)],
                        xa[0:65, 1, :],
                        start=False,
                        stop=True,
                    )
                qkT = wpool.tile([128, 4, N], BF, tag="qkT")
                nc.vector.tensor_copy(qkT[:], qkps[:])

                # ---- v matmuls (v in [m, c] layout) -> psum -> sbuf fp16
                vps = ps_sc.tile([128, 2, C], FP, tag="sc")
                for mt in range(2):
                    nc.tensor.matmul(
                        vps[:, mt, :],
                        xa[:, 0, 128 * mt : 128 * (mt + 1)],
                        wv_sb[:, 0, :],
                        start=True,
                        stop=False,
                    )
                    nc.tensor.matmul(
                        vps[:, mt, :],
                        xa[0:65, 1, 128 * mt : 128 * (mt + 1)],
                        wv_sb[0:65, 1, :],
                        start=False,
                        stop=True,
                    )
                vsb = wpool.tile([128, 2, C], BF, tag="vsb")
                nc.vector.tensor_copy(vsb[:], vps[:])

                # ---- scores S^T + exp, 3 chunks of 4 (h, mt) slabs
                es = bpool.tile([128, FREE], BF, tag="es")
                for ch in range(3):
                    scps = ps_sc.tile([128, 4, N], FP, tag="sc")
                    for kk in range(4):
                        h, mt = divmod(4 * ch + kk, 2)
                        qt, qr = q_loc[h]
                        kt, kr = k_loc[h]
                        nc.tensor.matmul(
                            scps[:, kk, :],
                            qkT[kr : kr + 32, kt, 128 * mt : 128 * (mt + 1)],
                            qkT[qr : qr + 32, qt, :],
                            start=True,
                            stop=True,
                            tile_position=(kr, 0),
                        )
                    nc.scalar.activation(
                        es[:, 1024 * ch : 1024 * (ch + 1)], scps[:], EXP
                    )

                # ---- P = exp(S^T) * EM, split DVE / GpSimd (chunked for overlap)
                p_t = bpool.tile([128, FREE], BF, tag="P")
                emj = em_sb[:, j, :]
                bounds = [0, PM_DVE] if PM_DVE > 0 else [0]
                if PM_DVE < FREE:
                    mid = (PM_DVE + FREE) // 2
                    mid -= mid % 512
                    if mid > PM_DVE:
                        bounds.append(mid)
                    bounds.append(FREE)
                for bi in range(len(bounds) - 1):
                    lo, hi = bounds[bi], bounds[bi + 1]
                    eng = nc.vector if hi <= PM_DVE else nc.gpsimd
                    eng.tensor_tensor(
                        p_t[:, lo:hi], es[:, lo:hi], emj[:, lo:hi], MUL
                    )

                # ---- flipped PV + denominator:  out [n, 6*(32+1)] per n-chunk
                pv = ps_py.tile([128, 2, 6, 33], FP, tag="py")
                for h in range(HEADS):
                    for nt in range(2):
                        # each accumulation group must complete before the next
                        # starts: a start=True matmul clears has_written for
                        # the whole PSUM bank
                        for mt in range(2):
                            o = 512 * h + 256 * mt + 128 * nt
                            nc.tensor.matmul(
                                pv[:, nt, h, 0:32],
                                p_t[:, o : o + 128],
                                vsb[:, mt, 32 * h : 32 * (h + 1)],
                                start=(mt == 0),
                                stop=(mt == 1),
                            )
                        for mt in range(2):
                            o = 512 * h + 256 * mt + 128 * nt
                            nc.tensor.matmul(
                                pv[:, nt, h, 32:33],
                                p_t[:, o : o + 128],
                                ones1[:],
                                start=(mt == 0),
                                stop=(mt == 1),
                            )

                # ---- normalize: aout[n, h, d] = pv * (1/den) broadcast over d
                ivd = wpool.tile([128, 2, 6], FP, tag="ivd")
                nc.vector.reciprocal_approx_fast(ivd[:], pv[:, :, :, 32])
                aout = wpool.tile([128, 2, 6, 32], BF, tag="aout")
                nc.vector.tensor_tensor(
                    aout[:],
                    pv[:, :, :, 0:32],
                    ivd[:].unsqueeze(3).broadcast_to([128, 2, 6, 32]),
                    MUL,
                )
                av = aout[:].rearrange("p t h d -> p t (h d)")

                # ---- transpose attn-out to [c, n]: DMA xbar (c 0:128) + PE (c 128:192)
                for nt in range(2):
                    nc.scalar.dma_start_transpose(
                        aoTa[asel][:, nt, :], av[:, nt, 0:128]
                    )
                tp = ps_py.tile([64, 2, 128], BF, tag="py")
                for nt in range(2):
                    nc.tensor.transpose(tp[:, nt, :], av[:, nt, 128:192], ident[:])
                nc.vector.tensor_copy(aoTb[asel][0:64, :, :], tp[:])

                # ---- output projection y[n, c] and store
                yps = ps_py.tile([128, 2, C], FP, tag="py")
                for nt in range(2):
                    nc.tensor.matmul(
                        yps[:, nt, :],
                        aoTa[asel][:, nt, :],
                        wp_sb[:, 0, :],
                        start=True,
                        stop=False,
                    )
                    nc.tensor.matmul(
                        yps[:, nt, :],
                        aoTb[asel][0:65, nt, :],
                        wp_sb[0:65, 1, :],
                        start=False,
                        stop=True,
                    )
                ysb = wpool.tile([128, 2, C], FP, tag="ysb")
                dbg = os.environ.get("DBG_STAGE", "")
                if not dbg:
                    nc.scalar.copy(ysb[:], yps[:])
                else:
                    nc.gpsimd.memset(ysb[:], 0.0)
                    if dbg == "es0":
                        nc.vector.tensor_copy(
                            ysb[:].rearrange("p a b -> p (a b)"), es[:, 0:384]
                        )
                    elif dbg == "p0":
                        nc.vector.tensor_copy(
                            ysb[:].rearrange("p a b -> p (a b)"), p_t[:, 0:384]
                        )
                    elif dbg == "pv":
                        nc.vector.tensor_copy(
                            ysb[:].rearrange("p a b -> p (a b)"),
                            pv[:, :, :, 0:32].rearrange("p t h d -> p (t h d)"),
                        )
                    elif dbg == "den":
                        nc.vector.tensor_copy(
                            ysb[:, 0, 0:12], pv[:, :, :, 32].rearrange("p t h -> p (t h)")
                        )
                    elif dbg == "aout":
                        nc.vector.tensor_copy(
                            ysb[:].rearrange("p a (h d) -> p a h d", h=6), aout[:]
                        )
                    elif dbg == "aoTa":
                        nc.vector.tensor_copy(ysb[:, :, 0:128], aoTa[asel][:])
                    elif dbg == "aoTb":
                        nc.vector.tensor_copy(ysb[:, :, 0:128], aoTb[asel][:])
                    elif dbg == "qkT":
                        nc.vector.tensor_copy(
                            ysb[:].rearrange("p a b -> p (a b)"),
                            qkT[:, 0:2, 0:192].rearrange("p a b -> p (a b)"),
                        )
                    elif dbg == "ivd":
                        nc.vector.tensor_copy(
                            ysb[:, 0, 0:12], ivd[:].rearrange("p t h -> p (t h)")
                        )
                    elif dbg == "vsb":
                        nc.vector.tensor_copy(ysb[:], vsb[:])
                nc.sync.dma_start(y_d[w], ysb[:])

            for rep in range(repeat):
                for it in range(WPC):
                    window(it)

    nc.compile()
    return nc


def _prep_inputs(inputs):
    x = np.asarray(inputs["x"], np.float32)
    mask = np.asarray(inputs["mask"], np.float32)
    w_qkv = np.asarray(inputs["w_qkv"], np.float32)
    b_qkv = np.asarray(inputs["b_qkv"], np.float32)
    w_proj = np.asarray(inputs["w_proj"], np.float32)
    b_proj = np.asarray(inputs["b_proj"], np.float32)
    H, W = int(inputs["H"]), int(inputs["W"])

    scale = float(D) ** -0.5
    rpb = _pos_bias_host(
        H,
        W,
        *[
            np.asarray(inputs[k], np.float32)
            for k in (
                "pw0",
                "pb0",
                "g1",
                "be1",
                "w1",
                "b1",
                "g2",
                "be2",
                "w2",
                "b2",
                "g3",
                "be3",
                "w3",
                "b3",
            )
        ],
    )

    # EM[mb, p, h*512 + mt*256 + n] = exp(mask[mb, n, m] + rpb[n, m, h]), m = mt*128+p
    bias = mask.transpose(0, 2, 1)[:, None] + rpb.transpose(2, 1, 0)[None]
    em = np.exp(bias)  # [64, 6, 256(m), 256(n)]
    em = em.reshape(NMASK, HEADS, 2, 128, N).transpose(0, 3, 1, 2, 4)
    em = np.ascontiguousarray(em.reshape(NMASK, 128, FREE)).astype(np.float16)

    # packed/augmented weights
    wq = np.vstack([w_qkv[:, 0:C] * scale, (b_qkv[0:C] * scale)[None]])  # [193, 192]
    wk = np.vstack([w_qkv[:, C : 2 * C], b_qkv[C : 2 * C][None]])
    mmdt = np.float16
    wqk = np.zeros((193, 512), np.float32)
    wqk[:, 0:128] = wq[:, 0:128]
    wqk[:, 128:256] = wk[:, 0:128]
    wqk[:, 256:320] = wq[:, 128:192]
    wqk[:, 384:448] = wk[:, 128:192]
    wqk = wqk.astype(mmdt)
    wv = np.ascontiguousarray(np.vstack([w_qkv[:, 2 * C :], b_qkv[2 * C :][None]])).astype(mmdt)
    wp = np.ascontiguousarray(np.vstack([w_proj, b_proj[None]])).astype(mmdt)
    ident = np.eye(128, dtype=mmdt)

    # per-core x^T with ones row, padded to [B, 128, 2, N] for 1-DMA loads
    xt_aug = np.zeros((B, 128, 2, N), mmdt)
    xT = x.transpose(0, 2, 1)  # [B, C, N]
    xt_aug[:, :, 0, :] = xT[:, 0:128, :]
    xt_aug[:, 0:64, 1, :] = xT[:, 128:192, :]
    xt_aug[:, 64, 1, :] = 1.0

    in_maps = []
    for core in range(NCORES):
        bs = [_win_to_b(core, w) for w in range(WPC)]
        in_maps.append(
            {
                "xt": np.ascontiguousarray(xt_aug[bs]),
                "em": np.ascontiguousarray(em[MPC * core : MPC * (core + 1)]),
                "wqk": wqk,
                "wv": wv,
                "wp": wp,
                "ident": ident,
            }
        )
    return in_maps


def _assemble(results):
    out = np.empty((B, N, C), np.float32)
    for core in range(NCORES):
        y = results[core]["y"]  # [WPC, 128, 2, C]
        for w in range(WPC):
            b = _win_to_b(core, w)
            out[b] = y[w].transpose(1, 0, 2).reshape(N, C)
    return out


def run(inputs, trace=False):
    from concourse.bass_utils import run_bass_kernel_spmd

    if "nc" not in _CACHE:
        _CACHE["nc"] = _build_nc()
    in_maps = _prep_inputs(inputs)
    res = run_bass_kernel_spmd(
        _CACHE["nc"],
        in_maps,
        core_ids=list(range(NCORES)),
        trace=trace,
        trace_cores=[0] if trace else None,
    )
    return _assemble(res.results), res


def get_nc():
    if "nc" not in _CACHE:
        _CACHE["nc"] = _build_nc()
    return _CACHE["nc"]


def kernel(**inputs):
    out, _ = run(inputs, trace=bool(int(os.environ.get("KERNEL_TRACE", "0"))))
    return out
